# revision 1
# baseline (speedup 1.0000x reference)
"""Trainium2 Bass kernel for nn_BaselineMNISTClassifier (vq_codebook).

reference:
    x = samples - 0.5                        # [B, F]
    hv = einsum('bf,df->bd', x, bhv)         # [B, D]
    e = (hv > 0)                             # binary
    ham[b, c] = sum_d |e - centroids[c, d]|  # [B, C]
    return -ham

Identity used on device: with e' = (hv > 0) - 0.5 in {-1/2, +1/2} and
cmod = 1 - 2c in {-1, +1}:  |e - c| = e' * cmod + 1/2, so
    ham[b, c] = sum_d e'[b, d] * cmod[c, d] + D/2
which turns the broadcast Hamming into a second (tiny) matmul over the
same d-tiles.

Sharding: the D axis (10000) splits across 8 cores, 1250 (zero-padded
to 1280) per core. Every core sees the full batch and computes a
partial hamming [C, B]; the partials sum on the host (padded dims
contribute exactly 0: the centroid pad value 0.5 makes cmod = 0 there).

The encode matmul runs in float32r (~tf32 precision; streams one
column per cycle at N=512, measured 227 ns / matmul warm). Both
operands are host-transposed so the contraction dim F sits on SBUF
partitions; no on-device transposes anywhere. The hamming matmul runs
in bf16 (e', cmod are exact in bf16), so the device output is exact
integer arithmetic given the encode bits.

Perf structure (per core, measured ~181 us on hardware):
  - 52 warmup matmuls on dummy data release the PE HAM clock gate
    (1.2 -> 2.4 GHz) while the inputs stream in; the clock then stays
    warm for the whole kernel
  - input tiles are single-assignment (no slot reuse), so input DMAs
    never carry data-dependency waits; x triggers issue from SP, w and
    centroid/output triggers from GpSimd (each DMA trigger costs
    ~0.6 us of issue time on its engine)
  - fi-outer / bb-inner matmul order: 4 consecutive matmuls share the
    stationary weights, hiding the fused fp32r LDWEIGHTS
  - all four hamming accumulators of a b-group live in ONE PSUM bank
    at partition offsets 0/32/64/96 via col-tiled matmuls
    (tile_position), which frees 7 PSUM banks for the encode
    accumulation (deep multi-buffering, no start-of-group stalls)
  - hamming matmuls are emitted one d-tile late so the PE never waits
    on the DVE binarize; the epilogue alternates Scalar/DVE and each
    output block DMAs out as soon as its accumulation closes

Toolchain notes: built on bacc.Bacc (its compile() legalizes the
1-sync-wait-per-instruction hardware limit via event semaphores, which
raw Bass + TileContext does not); output DMAs go through nc.gpsimd
because SP DMA_DIRECT2D triggers only take a single wait.
"""

import sys

sys.path.insert(0, "/opt/trn_rl_repo")

import numpy as np

import concourse.bacc as bacc
import concourse.bass as bass
import concourse.mybir as mybir
import concourse.tile as tile
from concourse.bass_utils import run_bass_kernel_spmd

B = 4096
F = 784
D = 10000
C = 10
NCORES = 8
DREAL = D // NCORES          # 1250 real dims per core
DP = 1280                    # padded to 10 d-tiles of 128
ND = DP // 128               # 10
NB = B // 512                # 8 b-blocks of 512
FT = [(i * 128, min(128, F - i * 128)) for i in range((F + 127) // 128)]
NF = len(FT)                 # 7 (6x128 + 16)
NWARM = 52                   # PE warmup matmuls
NFILL = 5                    # dummy matmuls per fi-step of the first group

F32 = mybir.dt.float32
F32R = mybir.dt.float32r
BF16 = mybir.dt.bfloat16
OP = mybir.AluOpType
AF = mybir.ActivationFunctionType

_NC_CACHE = {}


def _build_nc():
    if "nc" in _NC_CACHE:
        return _NC_CACHE["nc"]
    nc = bacc.Bacc("TRN2", debug=False, target_bir_lowering=False)
    xT = nc.dram_tensor("xT", [F, B], F32R, kind="ExternalInput")
    wT = nc.dram_tensor("wT", [F, DP], F32R, kind="ExternalInput")
    cT = nc.dram_tensor("cT", [DP, C], F32, kind="ExternalInput")
    out = nc.dram_tensor("out", [C, B], F32, kind="ExternalOutput")

    with tile.TileContext(nc) as tc:
        with (
            tc.tile_pool(name="dum", bufs=2) as dumpool,
            tc.tile_pool(name="xp", bufs=NB // 2 * NF) as xpool,
            tc.tile_pool(name="wp", bufs=(ND + 1) // 2 * NF) as wpool,
            tc.tile_pool(name="cp", bufs=1) as cpool,
            tc.tile_pool(name="cmp", bufs=1) as cmpool,
            tc.tile_pool(name="ep", bufs=8) as epool,
            tc.tile_pool(name="op", bufs=4) as opool,
            tc.tile_pool(name="pse", bufs=7, space="PSUM") as psepool,
            tc.tile_pool(name="ps2", bufs=1, space="PSUM") as ps2pool,
        ):
            # --- PE warmup: release the HAM clock gate while inputs load.
            wdum = dumpool.tile([128, 128], BF16)
            nc.gpsimd.memset(wdum[:], 1.0)
            xdum = dumpool.tile([128, 512], BF16)
            nc.gpsimd.memset(xdum[:], 1.0)
            psdum = psepool.tile([128, 512], F32, name="psdum", tag="pse")
            for i in range(NWARM):
                nc.tensor.matmul(psdum[:], wdum[:], xdum[:],
                                 start=(i == 0), stop=(i == NWARM - 1))

            # --- centroid prep: one DMA for all 10 d-tiles, then
            # cmod = 1 - 2c (bf16). Pad rows are 0.5 -> cmod = 0.
            ct = cpool.tile([128, ND * C], F32)
            nc.gpsimd.dma_start(
                ct[:].rearrange("p (a c) -> p a c", c=C),
                cT.ap().rearrange("(a p) c -> p a c", p=128))
            cmod = cmpool.tile([128, ND * C], BF16)
            nc.scalar.activation(cmod[:], ct[:], AF.Copy, bias=1.0,
                                 scale=-2.0)
            cmods = [cmod[:, di * C:(di + 1) * C] for di in range(ND)]

            # --- input loads; tiles single-assignment (loaded once, no
            # slot reuse) so input DMAs never carry data waits. x tiles
            # span two b-blocks, w tiles two d-tiles.
            xts = {}
            wts = {}

            def load_x(bp, fi):   # bp = b-block pair index (0..3)
                f0, fl = FT[fi]
                xt = xpool.tile([fl, 1024], F32R, name=f"xt_{bp}_{fi}",
                                tag="xt")
                nc.sync.dma_start(
                    xt[:], xT[f0:f0 + fl, bp * 1024:(bp + 1) * 1024])
                # center (x - 0.5) in place on DVE
                nc.vector.tensor_scalar_add(xt[:], xt[:], -0.5)
                xts[bp, fi] = xt

            def load_w(dp, fi):   # dp = d-tile pair index (0..4)
                f0, fl = FT[fi]
                wid = min(256, DP - dp * 256)
                wt = wpool.tile([fl, wid], F32R, name=f"wt_{dp}_{fi}",
                                tag="wt")
                nc.gpsimd.dma_start(
                    wt[:], wT[f0:f0 + fl, dp * 256:dp * 256 + wid])
                wts[dp, fi] = wt

            for i in range(5):
                for fi in range(NF):
                    if i < 4:
                        load_x(i, fi)
                    load_w(i, fi)

            def xop(bb, fi):
                return xts[bb // 2, fi][:, (bb % 2) * 512:(bb % 2 + 1) * 512]

            def wop(di, fi):
                return wts[di // 2, fi][:, (di % 2) * 128:(di % 2 + 1) * 128]

            # --- main compute: two b-groups of 4 blocks.
            for bg in range(2):
                bbs = list(range(bg * 4, bg * 4 + 4))
                ps2 = ps2pool.tile([128, 512], F32, name=f"ps2_{bg}",
                                   tag="ps2")
                psum2 = {bb: ps2[32 * (bb % 4):32 * (bb % 4) + C, :]
                         for bb in bbs}
                pending = []
                for di in range(ND):
                    pses = {}
                    for bb in bbs:
                        pses[bb] = psepool.tile([128, 512], F32,
                                                name=f"pse_{di % 2}_{bb}",
                                                tag="pse")
                    for fi in range(NF):
                        for bb in bbs:
                            nc.tensor.matmul(pses[bb][:], wop(di, fi),
                                             xop(bb, fi),
                                             start=(fi == 0),
                                             stop=(fi == NF - 1))
                    ets = {}
                    for bb in bbs:
                        # e' = (hv > 0) - 0.5 in {-1/2, +1/2}; the last
                        # d-tile binarizes in halves so its hamming
                        # matmuls overlap the binarize (no encode work
                        # left to hide the chain behind)
                        et = epool.tile([128, 512], BF16,
                                        name=f"et_{di % 2}_{bb}", tag="et")
                        if di == ND - 1:
                            for h in range(2):
                                sl = slice(h * 256, (h + 1) * 256)
                                nc.vector.tensor_scalar(
                                    et[:, sl], pses[bb][:, sl], 0.0, 0.5,
                                    op0=OP.is_gt, op1=OP.subtract)
                        else:
                            nc.vector.tensor_scalar(et[:], pses[bb][:],
                                                    0.0, 0.5,
                                                    op0=OP.is_gt,
                                                    op1=OP.subtract)
                        ets[bb] = et
                    for pdi, pbb, pet in pending:
                        nc.tensor.matmul(psum2[pbb], cmods[pdi],
                                         pet[:], start=(pdi == 0),
                                         stop=(pdi == ND - 1),
                                         tile_position=(0, 32 * (pbb % 4)))
                    pending = [(di, bb, ets[bb]) for bb in bbs]
                for pdi, pbb, pet in pending:
                    for h in range(2):
                        sl = slice(h * 256, (h + 1) * 256)
                        nc.tensor.matmul(psum2[pbb][:, sl], cmods[pdi],
                                         pet[:, sl], start=(pdi == 0),
                                         stop=(pdi == ND - 1),
                                         tile_position=(0, 32 * (pbb % 4)))
                    # out = -(psum2 + DREAL/2); alternate engines so the
                    # four epilogues drain in parallel
                    ot = opool.tile([C, 512], F32, name=f"ot_{pbb % 4}",
                                    tag="ot")
                    if pbb % 2 == 0:
                        nc.scalar.activation(ot[:], psum2[pbb], AF.Copy,
                                             bias=-float(DREAL) / 2.0,
                                             scale=-1.0)
                    else:
                        nc.vector.tensor_scalar(ot[:], psum2[pbb],
                                                float(DREAL) / 2.0, -1.0,
                                                op0=OP.add, op1=OP.mult)
                    nc.gpsimd.dma_start(
                        out[:, pbb * 512:(pbb + 1) * 512], ot[:])
    nc.compile()
    _NC_CACHE["nc"] = nc
    return nc


def _prep_in_maps(samples, bhv_matrix, centroids):
    samples = np.ascontiguousarray(samples, dtype=np.float32)
    bhv_matrix = np.ascontiguousarray(bhv_matrix, dtype=np.float32)
    centroids = np.ascontiguousarray(centroids, dtype=np.float32)
    xT = np.ascontiguousarray(samples.T)  # [F, B]
    in_maps = []
    for k in range(NCORES):
        lo_, hi_ = k * DREAL, (k + 1) * DREAL
        wTk = np.zeros((F, DP), dtype=np.float32)
        wTk[:, :DREAL] = bhv_matrix[lo_:hi_, :].T
        cTk = np.full((DP, C), 0.5, dtype=np.float32)
        cTk[:DREAL, :] = centroids[:, lo_:hi_].T
        in_maps.append({"xT": xT, "wT": wTk, "cT": cTk})
    return in_maps


def _run(samples, bhv_matrix, centroids, **spmd_kwargs):
    nc = _build_nc()
    in_maps = _prep_in_maps(samples, bhv_matrix, centroids)
    res = run_bass_kernel_spmd(nc, in_maps, core_ids=list(range(NCORES)),
                               **spmd_kwargs)
    acc = np.zeros((C, B), dtype=np.float32)
    for r in res.results:
        acc += r["out"]
    return np.ascontiguousarray(acc.T), res


def kernel(samples, bhv_matrix, centroids):
    out, _ = _run(samples, bhv_matrix, centroids)
    return out



# revision 7
# speedup vs baseline: 1.7058x; 1.7058x over previous
"""Trainium2 Bass kernel for nn_BaselineMNISTClassifier (vq_codebook).

reference:
    x = samples - 0.5                        # [B, F]
    hv = einsum('bf,df->bd', x, bhv)         # [B, D]
    e = (hv > 0)                             # binary
    ham[b, c] = sum_d |e - centroids[c, d]|  # [B, C]
    return -ham

Identity used on device: with e' = (hv > 0) - 0.5 in {-1/2, +1/2} and
cmod = 1 - 2c in {-1, +1}:  |e - c| = e' * cmod + 1/2, so
    ham[b, c] = sum_d e'[b, d] * cmod[c, d] + D/2
which turns the broadcast Hamming into a second (tiny) matmul over the
same d-tiles.

Sharding: the D axis (10000) splits across 8 cores, 1250 (zero-padded
to 1280) per core. Every core sees the full batch and computes a
partial hamming [C, B]; the partials sum on the host (padded dims
contribute exactly 0: the centroid pad value 0.5 makes cmod = 0 there).

This version runs both matmuls in fp8e4 with MatmulPerfMode.DoubleRow
(2x PE throughput vs bf16/fp32r; contraction = 2x128 = 256 per matmul
via stacked k-tiles in the free dim: ins are [128, 2, N], result =
sum_i W[:, i].T @ X[:, i]).

  - encode: x - 0.5 and bhv quantize to fp8e4 on the host (measured
    rel err 8e-3 vs the 2e-2 gate; hv std is 4.7 so the ~0.1 quant
    noise flips ~90 of 10000 sign bits per sample). F=784 pads to
    1024 = 4 k-groups of 256. b-chunks of 256 (DoubleRow moving free
    dim is 2N <= 512), 4 chunks per group so 4 consecutive matmuls
    share stationary weights.
  - binarize: chunks 0,1 on Scalar (Sign(hv) -> +-1), chunks 2,3 on
    DVE ((hv>0)-0.5 -> +-0.5); both write fp8e4 directly into the
    paired-k-tile layout the hamming DoubleRow matmul wants. Two
    centroid tiles bake the matching scale per chunk ((1-2c)/2 for
    Sign chunks, 1-2c for DVE chunks), so every real-dim product is
    +-1/2 exactly and the hamming stays integer-exact given the bits.
  - hamming: DoubleRow over d-tile pairs: lhsT = cmod [128, 2, C],
    moving = e' [128, 2, 256], 5 matmuls per (group, chunk).

PSUM: ZERO_REGION granularity is a full 2KB bank row and dual-fp8
matmuls only accept dst partition base 0, so every concurrent
accumulator owns a bank: 4 encode chunk banks (ring) + 4 hamming
chunk banks = 8.

Perf structure follows the fp32r baseline: warmup matmuls release the
PE HAM clock gate while inputs stream in; input tiles are
single-assignment; hamming for pair p is emitted after encode of
d-tile 2p+2 (one pair late) so the PE never waits on the binarize;
each group's last-pair hamming flushes after the NEXT group's first
encode d-tile, and the epilogue alternates Scalar/DVE per chunk.
"""

import sys

sys.path.insert(0, "/opt/trn_rl_repo")

import numpy as np

import concourse.bacc as bacc
import concourse.bass as bass
import concourse.mybir as mybir
import concourse.tile as tile
from concourse.bass_utils import run_bass_kernel_spmd

B = 4096
F = 784
D = 10000
C = 10
NCORES = 8
DREAL = D // NCORES          # 1250 real dims per core
DP = 1280                    # padded to 10 d-tiles of 128
ND = DP // 128               # 10
KG = 4                       # k-groups of 256 (F zero-padded to 1024)
FP = 256 * KG                # 1024
NBG = 4                      # b chunk-groups (1024 samples each)
NCH = 4                      # chunks of 256 per group
NWARM = 52                   # PE warmup matmuls
CP = 16                      # classes padded to 16 for dual-fp8 ldweights

F32 = mybir.dt.float32
F8 = mybir.dt.float8e4
OP = mybir.AluOpType
AF = mybir.ActivationFunctionType
DROW = mybir.MatmulPerfMode.DoubleRow

_NC_CACHE = {}


def _build_nc():
    if "nc" in _NC_CACHE:
        return _NC_CACHE["nc"]
    nc = bacc.Bacc("TRN2", debug=False, target_bir_lowering=False)
    x8 = [nc.dram_tensor(f"x8_{g}", [128, 2, B], F8, kind="ExternalInput")
          for g in range(KG)]
    w8 = [nc.dram_tensor(f"w8_{g}", [128, 2, DP], F8, kind="ExternalInput")
          for g in range(KG)]
    cT = nc.dram_tensor("cT", [DP, C], F32, kind="ExternalInput")
    out = nc.dram_tensor("out", [C, B], F32, kind="ExternalOutput")

    with tile.TileContext(nc) as tc:
        with (
            tc.tile_pool(name="dum", bufs=2) as dumpool,
            tc.tile_pool(name="xp", bufs=KG * NBG) as xpool,
            tc.tile_pool(name="wp", bufs=KG * ND // 2) as wpool,
            tc.tile_pool(name="cp", bufs=1) as cpool,
            tc.tile_pool(name="chp", bufs=2) as champool,
            tc.tile_pool(name="ep", bufs=12) as epool,
            tc.tile_pool(name="op", bufs=4) as opool,
            tc.tile_pool(name="pse", bufs=4, space="PSUM") as psepool,
            tc.tile_pool(name="ps2", bufs=4, space="PSUM") as ps2pool,
        ):
            # --- PE warmup: release the HAM clock gate while inputs load.
            wdum = dumpool.tile([128, 2, 128], F8)
            nc.gpsimd.memset(wdum[:, :, :], 1.0)
            xdum = dumpool.tile([128, 2, 256], F8)
            nc.gpsimd.memset(xdum[:, :, :], 1.0)
            psdum = psepool.tile([128, 512], F32, name="psdum", tag="pse")
            for i in range(NWARM):
                nc.tensor.matmul(psdum[:, 0:256], wdum[:, :, :],
                                 xdum[:, :, :], start=(i == 0),
                                 stop=(i == NWARM - 1), perf_mode=DROW)

            # --- centroid prep: one DMA for all 10 d-tiles. Two scaled
            # copies: chunks 0,1 binarize on Scalar (Sign -> +-1) and use
            # cmod = (1-2c)/2; chunks 2,3 binarize on DVE (-> +-1/2) and
            # use cmod = 1-2c. Pad rows are 0.5 -> cmod = 0 either way.
            # Class columns pad 10 -> 16 with zeros: dual-fp8 LDWEIGHTS
            # requires the k-tile stride to be a multiple of 16 (walrus
            # s3_lw_dual_fp8_restrictions).
            ct = cpool.tile([128, ND * C], F32)
            nc.gpsimd.dma_start(
                ct[:].rearrange("p (a c) -> p a c", c=C),
                cT.ap().rearrange("(a p) c -> p a c", p=128))
            cham_h = champool.tile([128, ND * CP], F8, name="cham_h")
            cham_f = champool.tile([128, ND * CP], F8, name="cham_f")
            nc.gpsimd.memset(cham_h[:], 0.0)
            nc.gpsimd.memset(cham_f[:], 0.0)
            for a in range(ND):
                src = ct[:, a * C:(a + 1) * C]
                nc.scalar.activation(cham_h[:, a * CP:a * CP + C], src,
                                     AF.Copy, bias=0.5, scale=-1.0)
                nc.scalar.activation(cham_f[:, a * CP:a * CP + C], src,
                                     AF.Copy, bias=1.0, scale=-2.0)
            cham3 = [t[:].rearrange("p (a c) -> p a c", c=CP)
                     for t in (cham_h, cham_h, cham_f, cham_f)]

            # --- input loads; tiles single-assignment (loaded once, no
            # slot reuse) so input DMAs never carry data waits.
            xts = {}
            wts = {}
            for mp in range(ND // 2):
                for g in range(KG):
                    t = wpool.tile([128, 2, 256], F8, name=f"wt_{g}_{mp}",
                                   tag="wt")
                    nc.gpsimd.dma_start(
                        t[:, :, :], w8[g][:, :, mp * 256:(mp + 1) * 256])
                    wts[g, mp] = t
            for bg in range(NBG):
                for g in range(KG):
                    t = xpool.tile([128, 2, 1024], F8, name=f"xt_{g}_{bg}",
                                   tag="xt")
                    nc.sync.dma_start(
                        t[:, :, :], x8[g][:, :, bg * 1024:(bg + 1) * 1024])
                    xts[g, bg] = t

            def enc_w(g, m):
                return wts[g, m // 2][:, :, (m % 2) * 128:(m % 2 + 1) * 128]

            def enc_x(g, bg, ch):
                return xts[g, bg][:, :, ch * 256:(ch + 1) * 256]

            def emit_ham(ps2t, p, ets_p):
                # dual-fp8 matmul dst partition base must be 0: one bank
                # per chunk accumulator
                for ch in range(NCH):
                    nc.tensor.matmul(ps2t[ch][0:CP, 0:256],
                                     cham3[ch][:, 2 * p:2 * p + 2, :],
                                     ets_p[ch][:, :, :],
                                     start=(p == 0), stop=(p == ND // 2 - 1),
                                     perf_mode=DROW)

            def emit_epi(ps2t, bg, ch, engine):
                ot = opool.tile([C, 256], F32, name=f"ot_{ch}", tag="ot")
                src = ps2t[ch][0:C, 0:256]
                if engine == "scalar":
                    nc.scalar.activation(ot[:], src, AF.Copy,
                                         bias=-DREAL / 2.0, scale=-1.0)
                else:
                    nc.vector.tensor_scalar(ot[:], src, DREAL / 2.0, -1.0,
                                            op0=OP.add, op1=OP.mult)
                chg = bg * NCH + ch
                nc.gpsimd.dma_start(
                    out[:, chg * 256:(chg + 1) * 256], ot[:])

            # --- main compute: 4 chunk-groups of 4 chunks (256 b each).
            prev_tail = None     # (ps2t, ets of pair 4, bg) from last group
            for bg in range(NBG):
                ps2t = [ps2pool.tile([128, 512], F32,
                                     name=f"ps2_{ch}", tag="ps2")
                        for ch in range(NCH)]
                ets = {}
                for m in range(ND):
                    pse_ch = [psepool.tile([128, 512], F32,
                                           name=f"pse_{ch}", tag="pse")
                              for ch in range(NCH)]
                    for g in range(KG):
                        for ch in range(NCH):
                            nc.tensor.matmul(pse_ch[ch][:, 0:256],
                                             enc_w(g, m), enc_x(g, bg, ch),
                                             start=(g == 0),
                                             stop=(g == KG - 1),
                                             perf_mode=DROW)
                    # flush delayed hamming now that the PE has a d-tile
                    # of encode work queued ahead of it
                    if m == 0 and prev_tail is not None:
                        pt_ps2, pt_ets, pt_bg = prev_tail
                        emit_ham(pt_ps2, ND // 2 - 1, pt_ets)
                        for ch in range(NCH):
                            emit_epi(pt_ps2, pt_bg, ch,
                                     "scalar" if ch % 2 == 0 else "vector")
                        prev_tail = None
                    if m >= 2 and m % 2 == 0:
                        p = m // 2 - 1
                        emit_ham(ps2t, p, [ets[p, ch] for ch in range(NCH)])
                    # binarize: chunks 0,1 on Scalar (Sign), 2,3 on DVE
                    for ch in range(NCH):
                        if (m // 2, ch) not in ets:
                            ets[m // 2, ch] = epool.tile(
                                [128, 2, 256], F8,
                                name=f"et_{m // 2}_{ch}", tag="et")
                        dst = ets[m // 2, ch][:, m % 2, :]
                        src = pse_ch[ch][:, 0:256]
                        if ch < 2:
                            nc.scalar.activation(dst, src, AF.Sign)
                        else:
                            nc.vector.tensor_scalar(dst, src, 0.0, 0.5,
                                                    op0=OP.is_gt,
                                                    op1=OP.subtract)
                last_p = ND // 2 - 1
                prev_tail = (ps2t, [ets[last_p, ch] for ch in range(NCH)],
                             bg)
            # final group's tail: no encode work left to hide behind, so
            # drain chunk by chunk (epilogue on DVE first; Scalar is busy
            # with the m=9 Sign binarize)
            pt_ps2, pt_ets, pt_bg = prev_tail
            for ch in range(NCH):
                nc.tensor.matmul(pt_ps2[ch][0:CP, 0:256],
                                 cham3[ch][:, 2 * (ND // 2 - 1):2 * (ND // 2), :],
                                 pt_ets[ch][:, :, :],
                                 start=False, stop=True,
                                 perf_mode=DROW)
                emit_epi(pt_ps2, pt_bg, ch,
                         "vector" if ch % 2 == 0 else "scalar")
    nc.compile()
    _NC_CACHE["nc"] = nc
    return nc


def _prep_in_maps(samples, bhv_matrix, centroids):
    f8 = mybir.dt.np(F8)
    samples = np.ascontiguousarray(samples, dtype=np.float32)
    bhv_matrix = np.ascontiguousarray(bhv_matrix, dtype=np.float32)
    centroids = np.ascontiguousarray(centroids, dtype=np.float32)

    xp = np.zeros((FP, B), dtype=np.float32)
    xp[:F, :] = samples.T - 0.5
    xq = xp.astype(f8).reshape(KG, 2, 128, B)
    x_maps = {f"x8_{g}": np.ascontiguousarray(xq[g].transpose(1, 0, 2))
              for g in range(KG)}

    in_maps = []
    for k in range(NCORES):
        lo_, hi_ = k * DREAL, (k + 1) * DREAL
        wp = np.zeros((FP, DP), dtype=np.float32)
        wp[:F, :DREAL] = bhv_matrix[lo_:hi_, :].T
        wq = wp.astype(f8).reshape(KG, 2, 128, DP)
        m = dict(x_maps)
        for g in range(KG):
            m[f"w8_{g}"] = np.ascontiguousarray(wq[g].transpose(1, 0, 2))
        cTk = np.full((DP, C), 0.5, dtype=np.float32)
        cTk[:DREAL, :] = centroids[:, lo_:hi_].T
        m["cT"] = cTk
        in_maps.append(m)
    return in_maps


def _run(samples, bhv_matrix, centroids, **spmd_kwargs):
    nc = _build_nc()
    in_maps = _prep_in_maps(samples, bhv_matrix, centroids)
    res = run_bass_kernel_spmd(nc, in_maps, core_ids=list(range(NCORES)),
                               **spmd_kwargs)
    acc = np.zeros((C, B), dtype=np.float32)
    for r in res.results:
        acc += r["out"]
    return np.ascontiguousarray(acc.T), res


def kernel(samples, bhv_matrix, centroids):
    out, _ = _run(samples, bhv_matrix, centroids)
    return out
